# revision 24
# baseline (speedup 1.0000x reference)
"""Trainium2 Bass kernel for nn_AttentionBlock (B=2, S=2048, D=1024, H=16, HD=64).

Sharding: 8 cores = 2 batches x 4 head-groups (4 heads each).
Each core computes, for its (batch b, head-group g):
  - fused QK projection (RoPE'd, feature-transposed layout) + V projection
  - causal attention for its 4 heads (scores computed transposed, softmax
    denominator via an appended ones-column in the PV matmul)
  - a PARTIAL output projection: x_g @ W_out[:, d-slice].T  -> [S, D] partial
The host sums the 4 partials per batch (linear unshard step) - no on-device
collectives needed.

Storage dtype is bf16 for all streamed operands (inputs, weights, rope
tables, activations); all matmul accumulation and the softmax are fp32 in
PSUM; the output is fp32.

Self-contained: hardcodes all shapes; imports only concourse + numpy.
"""
import math

import numpy as np

import concourse.bass as bass  # noqa: F401
import concourse.bacc as bacc
import concourse.mybir as mybir
import concourse.tile as tile
from concourse import bass_utils
from concourse.masks import make_upper_triangular

B, S, D, H = 2, 2048, 1024, 16
HD = D // H            # 64
G = 4                  # head-groups (cores per batch)
HPG = H // G           # 4 heads per group
N_CORES = 8
ROPE_BASE = 10000.0
F32 = mybir.dt.float32
F32R = mybir.dt.float32r
BF16 = mybir.dt.bfloat16

KT = S // 128          # 16 k-tiles of 128
ST = S // 128          # 16 s-tiles
DT = D // 128          # 8 d-chunks
SCALE = 1.0 / math.sqrt(HD)


def r(ap):
    """bitcast an f32 AP to float32r for full-rate PE matmul."""
    return ap.bitcast(F32R)


def sub512(off, n):
    """split (off, n) into 512-bank-aligned sub-chunks."""
    out = []
    end = off + n
    while off < end:
        m = min(512 - off % 512, end - off)
        out.append((off, m))
        off += m
    return out


def build_nc():
    nc = bacc.Bacc("TRN2", target_bir_lowering=False, debug=False,
                   num_devices=N_CORES)

    xT = nc.dram_tensor("xT", [D, S], BF16, kind="ExternalInput").ap()
    # QK weights, transposed+permuted: columns = [QA|QB|KA|KB] of 128 each.
    wqkT = nc.dram_tensor("wqkT", [D, 512], BF16, kind="ExternalInput").ap()
    # V weights, transposed: columns = 4 heads x 64 feats.
    wvT = nc.dram_tensor("wvT", [D, 256], BF16, kind="ExternalInput").ap()
    # RoPE tables, [128, S]: rows = 4x (32 freqs).
    cos4 = nc.dram_tensor("cos4", [128, S], F32, kind="ExternalInput").ap()
    sin4 = nc.dram_tensor("sin4", [128, S], F32, kind="ExternalInput").ap()
    # W_out columns for this group's features, transposed: [256, D].
    woT = nc.dram_tensor("woT", [256, D], BF16, kind="ExternalInput").ap()
    out = nc.dram_tensor("out", [S, D], F32, kind="ExternalOutput").ap()

    with tile.TileContext(nc) as tc:
        _body(nc, tc, xT, wqkT, wvT, cos4, sin4, woT, out)
    nc.compile()
    return nc


def _attention_head(nc, attps, ptp, h, qlo, qhi, xacc, kc, qc, v_t, triu):
    """Emit QK->exp->mask->PV for one head over q range [qlo, qhi),
    qhi - qlo <= 512 and 512-aligned (single psum bank)."""
    hp, hi = divmod(h, 2)
    hs = slice(64 * hi, 64 * hi + 64)
    t_end = qhi // 128
    for t in range(t_end):
        ktl = slice(t * 128, (t + 1) * 128)
        off = max(qlo, 128 * t)
        n = qhi - off
        base = off % 512
        sps = attps.tile([128, 512], F32, tag="sps")
        nc.tensor.matmul(
            sps[:, base:base + n], kc[hp][hs, ktl], qc[hp][hs, off:off + n],
            start=True, stop=True)
        pt = ptp.tile([128, 512], BF16, tag="pt")
        nc.scalar.activation(
            pt[:, base:base + n], sps[:, base:base + n],
            mybir.ActivationFunctionType.Exp, scale=SCALE)
        if off == 128 * t:
            # diagonal block: causal 0/1 mask (GpSimd - DVE is busier)
            nc.gpsimd.tensor_tensor(
                pt[:, base:base + 128], pt[:, base:base + 128],
                triu[:], mybir.AluOpType.mult)
        nc.tensor.matmul(
            xacc[:, off - qlo:off - qlo + n],
            v_t[t][:, 65 * h:65 * h + 65],
            pt[:, base:base + n],
            start=(t == 0), stop=(t == t_end - 1))


def _body(nc, tc, xT, wqkT, wvT, cos4, sin4, woT, out):
    with tc.tile_pool(name="const", bufs=1) as constp, \
         tc.tile_pool(name="rot", bufs=1) as rotp, \
         tc.tile_pool(name="vsd", bufs=1) as vsd, \
         tc.tile_pool(name="xnorm", bufs=1) as xnp, \
         tc.tile_pool(name="wo", bufs=1) as wop:
        # long-lived tiles
        triu = constp.tile([128, 128], BF16)
        make_upper_triangular(nc, triu[:], val=1.0, diag=True)
        # head-contiguous rotated Q/K: qc[i] holds heads 2i, 2i+1 with each
        # head's 64 features (x1;x2) contiguous on partitions
        qc = [rotp.tile([128, S], BF16, name=f"qc_{i}") for i in range(2)]
        kc = [rotp.tile([128, S], BF16, name=f"kc_{i}") for i in range(2)]
        v_t = [vsd.tile([128, 260], BF16, name=f"v_{st}") for st in range(ST)]
        xn = [xnp.tile([128, S], BF16, name=f"xn_{hp}") for hp in range(2)]
        wo_t = [wop.tile([128, D], BF16, name=f"wo_{d2}") for d2 in range(2)]

        # ============ Phase A+B: projections ============
        with tc.tile_pool(name="xw", bufs=1) as xw:
            x_t = [xw.tile([128, S], BF16, name=f"x_{d}") for d in range(DT)]
            wqk_t = [xw.tile([128, 512], BF16, name=f"wqk_{d}") for d in range(DT)]
            wv_t = [xw.tile([128, 256], BF16, name=f"wv_{d}") for d in range(DT)]
            cos_t = xw.tile([128, S], F32)
            sin_t = xw.tile([128, S], F32)
            # issue DMAs in consumption order: d-chunk 0 first
            for d in range(DT):
                nc.sync.dma_start(wqk_t[d][:], wqkT[d * 128:(d + 1) * 128, :])
                nc.sync.dma_start(x_t[d][:], xT[d * 128:(d + 1) * 128, :])
                nc.sync.dma_start(wv_t[d][:], wvT[d * 128:(d + 1) * 128, :])
                if d == 1:
                    nc.sync.dma_start(cos_t[:], cos4[:])
                    nc.sync.dma_start(sin_t[:], sin4[:])
            for d2 in range(2):
                nc.sync.dma_start(wo_t[d2][:], woT[d2 * 128:(d2 + 1) * 128, :])

            # concurrently open psum pools so the scheduler can interleave
            # the QK-proj and V-proj matmul streams
            with tc.tile_pool(name="qkps", bufs=2, space="PSUM") as qkps, \
                 tc.tile_pool(name="vps", bufs=2, space="PSUM") as vps, \
                 tc.tile_pool(name="ropet", bufs=2) as ropet:
                # ---- QK projection + RoPE (A/B block layout) ----
                for sc in range(S // 512):
                    sl = slice(sc * 512, (sc + 1) * 512)
                    ps = {}
                    rot_c = {}
                    for e in range(4):  # QA, QB, KA, KB
                        p = qkps.tile([128, 512], F32, tag=f"qk{e % 2}")
                        for d in range(DT):
                            nc.tensor.matmul(
                                p[:], wqk_t[d][:, e * 128:(e + 1) * 128],
                                x_t[d][:, sl],
                                start=(d == 0), stop=(d == DT - 1))
                        ps[e] = p
                        if e % 2 == 1:
                            A, Bp = ps[e - 1], ps[e]
                            oA = ropet.tile([128, 512], BF16, tag=f"rc{e-1}",
                                            name=f"rc{e-1}_{sc}")
                            oB = ropet.tile([128, 512], BF16, tag=f"rc{e}",
                                            name=f"rc{e}_{sc}")
                            rot_c[e - 1], rot_c[e] = oA, oB
                            t1 = ropet.tile([128, 512], F32, tag="t1")
                            t2 = ropet.tile([128, 512], F32, tag="t2")
                            # oA = A*cos - B*sin ; oB = B*cos + A*sin
                            nc.vector.tensor_tensor(
                                t1[:], A[:], cos_t[:, sl], mybir.AluOpType.mult)
                            nc.vector.tensor_tensor(
                                t2[:], Bp[:], sin_t[:, sl], mybir.AluOpType.mult)
                            nc.vector.tensor_tensor(
                                oA[:], t1[:], t2[:], mybir.AluOpType.subtract)
                            nc.vector.tensor_tensor(
                                t1[:], Bp[:], cos_t[:, sl], mybir.AluOpType.mult)
                            nc.vector.tensor_tensor(
                                t2[:], A[:], sin_t[:, sl], mybir.AluOpType.mult)
                            nc.vector.tensor_tensor(
                                oB[:], t1[:], t2[:], mybir.AluOpType.add)
                    # scatter A/B halves into head-contiguous layout:
                    # head h x1 -> qc[h//2][64*(h%2):+32], x2 -> +32:+64
                    for (a_e, b_e, dsts) in ((0, 1, qc), (2, 3, kc)):
                        for h in range(HPG):
                            dt_ = dsts[h // 2]
                            po = 64 * (h % 2)
                            nc.sync.dma_start(
                                dt_[po:po + 32, sl],
                                rot_c[a_e][32 * h:32 * h + 32, :])
                            nc.sync.dma_start(
                                dt_[po + 32:po + 64, sl],
                                rot_c[b_e][32 * h:32 * h + 32, :])

                    # ---- V for this s range: [s, d-local] + ones columns --
                    for st in range(4 * sc, 4 * (sc + 1)):
                        pv = vps.tile([128, 256], F32, tag="vps")
                        stl = slice(st * 128, (st + 1) * 128)
                        for d in range(DT):
                            nc.tensor.matmul(
                                pv[:], x_t[d][:, stl], wv_t[d][:],
                                start=(d == 0), stop=(d == DT - 1))
                        # strided copy psum [128,(4,64)] -> v cols
                        nc.vector.tensor_copy(
                            v_t[st][:].rearrange("p (h f) -> p h f", h=4)[:, :, 0:64],
                            pv[:].rearrange("p (h f) -> p h f", h=4))
                        nc.gpsimd.memset(
                            v_t[st][:].rearrange("p (h f) -> p h f", h=4)[:, :, 64:65],
                            1.0)

        # ======== Phase C+D: attention + out-projection per q-quarter ====
        # psum: sps [128,512]x4 + xacc [65,512]x2 + ops [128,512]x2 = 8 banks
        with tc.tile_pool(name="attps", bufs=4, space="PSUM") as attps, \
             tc.tile_pool(name="xaccps", bufs=2, space="PSUM") as xaccps, \
             tc.tile_pool(name="ptp", bufs=4) as ptp, \
             tc.tile_pool(name="nrm", bufs=2) as nrmp, \
             tc.tile_pool(name="ops", bufs=2, space="PSUM") as ops, \
             tc.tile_pool(name="oout", bufs=3) as ooutp:
            for qh in range(4):
                qlo, qhi = 512 * qh, 512 * (qh + 1)
                for hp in range(2):
                    heads = (2 * hp, 2 * hp + 1)
                    xaccs = {}
                    for h in heads:
                        xaccs[h] = xaccps.tile([65, 512], F32, tag="xacc",
                                               name=f"xacc_{h}")
                        _attention_head(nc, attps, ptp, h, qlo, qhi,
                                        xaccs[h], kc, qc, v_t, triu)
                    for h in heads:
                        xacc = xaccs[h]
                        recip = nrmp.tile([1, 512], F32, tag="recip")
                        nc.vector.reciprocal(recip[:], xacc[64:65, :])
                        rb = nrmp.tile([64, 512], F32, tag="rb")
                        nc.gpsimd.partition_broadcast(rb[:], recip[:])
                        dst = xn[h // 2][64 * (h % 2):64 * (h % 2) + 64, :]
                        nc.vector.tensor_tensor(
                            dst[:, qlo:qhi], xacc[0:64, :], rb[:],
                            mybir.AluOpType.mult)

                # partial out-projection for this quarter's s-tiles
                for st in range(4 * qh, 4 * (qh + 1)):
                    stl = slice(st * 128, (st + 1) * 128)
                    for c in range(2):
                        po = ops.tile([128, 512], F32, tag="ops")
                        for d2 in range(2):
                            nc.tensor.matmul(
                                po[:], xn[d2][:, stl],
                                wo_t[d2][:, c * 512:(c + 1) * 512],
                                start=(d2 == 0), stop=(d2 == 1))
                        ot = ooutp.tile([128, 512], F32, tag="ot")
                        if c == 0:
                            nc.vector.tensor_copy(ot[:], po[:])
                        else:
                            nc.scalar.copy(ot[:], po[:])
                        nc.sync.dma_start(out[stl, c * 512:(c + 1) * 512],
                                          ot[:])


def _to_bf16(a):
    import ml_dtypes
    return np.ascontiguousarray(a.astype(ml_dtypes.bfloat16))


def prepare_in_maps(inputs, positions, W_in, W_out):
    """Build per-core input shards (all host-side numpy prep)."""
    inputs = np.ascontiguousarray(inputs, dtype=np.float32)
    W_in = np.ascontiguousarray(W_in, dtype=np.float32)
    W_out = np.ascontiguousarray(W_out, dtype=np.float32)
    positions = np.asarray(positions)

    inv_freq = 1.0 / (ROPE_BASE ** (np.arange(0, HD, 2, dtype=np.float32) / HD))
    in_maps = []
    for core in range(N_CORES):
        b, g = divmod(core, G)
        heads = [g * HPG + h for h in range(HPG)]

        xTb = inputs[b].T                                          # [D, S]

        # RoPE tables [128, S]: rows = 4 copies of the 32 freqs
        ang = positions[b].astype(np.float32)[None, :] * inv_freq[:, None]
        cos4 = np.tile(np.cos(ang), (4, 1)).astype(np.float32)
        sin4 = np.tile(np.sin(ang), (4, 1)).astype(np.float32)

        # QK weight blocks: QA/QB/KA/KB, each 128 rows (4 heads x 32)
        def rows(base_off):
            idx = []
            for h in heads:
                idx.extend(h * 3 * HD + base_off + f for f in range(32))
            return idx
        qk_idx = rows(0) + rows(32) + rows(64) + rows(96)
        wqkT = W_in[qk_idx].T                                      # [D, 512]

        v_idx = []
        for h in heads:
            v_idx.extend(h * 3 * HD + 2 * HD + f for f in range(HD))
        wvT = W_in[v_idx].T                                        # [D, 256]

        # W_out columns for this group's feature slice, transposed
        dsl = [h * HD + f for h in heads for f in range(HD)]
        woT = W_out[:, dsl].T                                      # [256, D]

        in_maps.append({
            "xT": _to_bf16(xTb), "wqkT": _to_bf16(wqkT),
            "wvT": _to_bf16(wvT), "cos4": cos4, "sin4": sin4, "woT": _to_bf16(woT),
        })
    return in_maps


def assemble_output(results):
    """Sum the 4 per-group partials for each batch."""
    out = np.zeros((B, S, D), dtype=np.float32)
    for core in range(N_CORES):
        b = core // G
        out[b] += results[core]["out"]
    return out


_NC_CACHE = {}


def get_nc():
    if "nc" not in _NC_CACHE:
        _NC_CACHE["nc"] = build_nc()
    return _NC_CACHE["nc"]


def kernel(inputs, positions, W_in, W_out):
    nc = get_nc()
    in_maps = prepare_in_maps(inputs, positions, W_in, W_out)
    res = bass_utils.run_bass_kernel_spmd(
        nc, in_maps, core_ids=list(range(N_CORES)))
    return assemble_output(res.results)
